# revision 1
# baseline (speedup 1.0000x reference)
"""Attention kernel for trn2: B=4, N=2048, DIM=512, HEADS=8, DIM_HEAD=64.

Sharding: head-parallel across 8 cores (core h computes head h for all 4
batches). Each core returns a partial [4, 2048, 512] output (its head's
contribution through W_out); the host sums the 8 partials.

Per-core pipeline (all matmuls float32r, 1 cycle/row):
  phase 1: qkv = x @ W_h (natural layout) -> rotary on q,k (DVE) ->
           PE-transpose q,k to [d, n] layout; v kept natural with an
           appended ones column.
  phase 2: S_T[k,q] = kT.T @ qT + biasT (identity-matmul accumulate);
           exp on ACT; PV matmul with lhsT=[v|1] gives head_out.T and
           row-sums in one PSUM tile; scale by 1/sum; W_out projection.
"""

import numpy as np

B, N, DIM = 4, 2048, 512
HEADS, DH = 8, 64
P = 128
DC = DIM // P          # 4 dim chunks
NCH = N // P           # 16 n chunks
QT = 512               # q tile in phase 2
NQT = N // QT          # 4
KC = N // P            # 16 k chunks

_CACHE = {}


def _build():
    import concourse.mybir as mybir
    import concourse.tile as tile
    from concourse import bacc

    F32 = mybir.dt.float32
    F32R = mybir.dt.float32r
    MULT = mybir.AluOpType.mult

    nc = bacc.Bacc(None, target_bir_lowering=False)

    xT4_d = nc.dram_tensor("xT4", [B, P, DC, N], F32R, kind="ExternalInput")
    w_d = nc.dram_tensor("w", [P, DC, 3 * DH], F32R, kind="ExternalInput")
    wout_d = nc.dram_tensor("wout", [P, DIM], F32R, kind="ExternalInput")
    biasT_d = nc.dram_tensor("biasT", [N, N], F32R, kind="ExternalInput")
    ident_d = nc.dram_tensor("ident", [P, P], F32R, kind="ExternalInput")
    zpad_d = nc.dram_tensor("zpad", [DH, N], F32R, kind="ExternalInput")
    onesv_d = nc.dram_tensor("onesv", [P, KC], F32R, kind="ExternalInput")
    ones1_d = nc.dram_tensor("ones1", [1, DH], F32R, kind="ExternalInput")
    cos_d = nc.dram_tensor("cos_e", [P, NCH, DH // 2], F32, kind="ExternalInput")
    sin_d = nc.dram_tensor("sin_e", [P, NCH, DH // 2], F32, kind="ExternalInput")
    out_d = nc.dram_tensor("out", [B, N, DIM], F32, kind="ExternalOutput")

    def r(ap):
        return ap.bitcast(F32R)

    with tile.TileContext(nc) as tc:
        with tc.tile_pool(name="const", bufs=1) as cp:
            w_t = cp.tile([P, DC, 3 * DH], F32R, tag="w")
            nc.sync.dma_start(w_t[:], w_d[:, :, :])
            wout_t = cp.tile([P, DIM], F32R, tag="wout")
            nc.sync.dma_start(wout_t[:], wout_d[:, :])
            ident_t = cp.tile([P, P], F32R, tag="ident")
            nc.sync.dma_start(ident_t[:], ident_d[:, :])
            ones1_t = cp.tile([1, DH], F32R, tag="ones1")
            nc.sync.dma_start(ones1_t[:], ones1_d[:, :])
            ho_t = cp.tile([P, QT], F32R, tag="ho")
            nc.sync.dma_start(ho_t[DH:P, :], zpad_d[:, 0:QT])
            cos_t = cp.tile([P, NCH, DH // 2], F32, tag="cos")
            nc.sync.dma_start(cos_t[:], cos_d[:, :, :])
            sin_t = cp.tile([P, NCH, DH // 2], F32, tag="sin")
            nc.sync.dma_start(sin_t[:], sin_d[:, :, :])

            qT_b = [cp.tile([P, N], F32R, tag=f"qT{b}", name=f"qT{b}") for b in range(B)]
            kT_b = [cp.tile([P, N], F32R, tag=f"kT{b}", name=f"kT{b}") for b in range(B)]
            v_b = [cp.tile([P, KC, DH + 1], F32R, tag=f"v{b}", name=f"v{b}") for b in range(B)]
            for b in range(B):
                nc.sync.dma_start(qT_b[b][DH:P, :], zpad_d[:, :])
                nc.sync.dma_start(kT_b[b][DH:P, :], zpad_d[:, :])
                nc.sync.dma_start(v_b[b][:, :, DH : DH + 1], onesv_d[:, :, None])

            # ---- phase 1: qkv projection + rotary + transpose ----
            with (
                tc.tile_pool(name="p1", bufs=3) as p1,
                tc.tile_pool(name="ps1", bufs=2, space="PSUM") as ps1,
                tc.tile_pool(name="pst", bufs=2, space="PSUM") as pst,
            ):
                for b in range(B):
                    for i in range(NCH):
                        xt = p1.tile([P, DC, P], F32R, tag="xt")
                        nc.sync.dma_start(
                            xt[:], xT4_d[b, :, :, i * P : (i + 1) * P]
                        )
                        qkv_ps = ps1.tile([P, 3 * DH], F32, tag="qkv")
                        for dc in range(DC):
                            nc.tensor.matmul(
                                qkv_ps[:],
                                lhsT=xt[:, dc],
                                rhs=w_t[:, dc],
                                start=(dc == 0),
                                stop=(dc == DC - 1),
                            )
                        ce = cos_t[:, i]
                        se = sin_t[:, i]
                        qr = p1.tile([P, DH], F32R, tag="qr")
                        kr = p1.tile([P, DH], F32R, tag="kr")
                        for src_off, dst in ((0, qr), (DH, kr)):
                            s3 = qkv_ps[:, src_off : src_off + DH].rearrange(
                                "p (a t) -> p a t", t=2
                            )
                            d3 = dst.rearrange("p (a t) -> p a t", t=2)
                            e_, o_ = s3[:, :, 0], s3[:, :, 1]
                            t1 = p1.tile([P, DH // 2], F32, tag="t1")
                            t2 = p1.tile([P, DH // 2], F32, tag="t2")
                            nc.vector.tensor_mul(t1[:], e_, ce)
                            nc.vector.tensor_mul(t2[:], o_, se)
                            nc.vector.tensor_sub(d3[:, :, 0], t1[:], t2[:])
                            t3 = p1.tile([P, DH // 2], F32, tag="t3")
                            t4 = p1.tile([P, DH // 2], F32, tag="t4")
                            nc.vector.tensor_mul(t3[:], o_, ce)
                            nc.vector.tensor_mul(t4[:], e_, se)
                            nc.vector.tensor_add(d3[:, :, 1], t3[:], t4[:])
                        nc.vector.tensor_copy(
                            v_b[b][:, i, 0:DH], qkv_ps[:, 2 * DH : 3 * DH]
                        )
                        qtp = pst.tile([DH, P], F32R, tag="qtp")
                        nc.tensor.transpose(qtp[:], qr[:], ident_t[:])
                        nc.vector.tensor_copy(
                            qT_b[b][0:DH, i * P : (i + 1) * P], qtp[:]
                        )
                        ktp = pst.tile([DH, P], F32R, tag="ktp")
                        nc.tensor.transpose(ktp[:], kr[:], ident_t[:])
                        nc.vector.tensor_copy(
                            kT_b[b][0:DH, i * P : (i + 1) * P], ktp[:]
                        )

            # ---- phase 2: attention ----
            with (
                tc.tile_pool(name="p2", bufs=3) as p2,
                tc.tile_pool(name="ps2", bufs=2, space="PSUM") as ps2,
                tc.tile_pool(name="pso", bufs=2, space="PSUM") as pso,
                tc.tile_pool(name="psw", bufs=2, space="PSUM") as psw,
            ):
                for b in range(B):
                    for jq in range(NQT):
                        qs = slice(jq * QT, (jq + 1) * QT)
                        outT_ps = pso.tile([DH + 1, QT], F32, tag="outT")
                        for kc in range(KC):
                            bt = p2.tile([P, QT], F32R, tag="bt")
                            nc.sync.dma_start(
                                bt[:], biasT_d[kc * P : (kc + 1) * P, qs]
                            )
                            s_ps = ps2.tile([P, QT], F32, tag="s")
                            nc.tensor.matmul(
                                s_ps[:],
                                lhsT=kT_b[b][:, kc * P : (kc + 1) * P],
                                rhs=qT_b[b][:, qs],
                                start=True,
                                stop=False,
                            )
                            nc.tensor.matmul(
                                s_ps[:],
                                lhsT=ident_t[:],
                                rhs=bt[:],
                                start=False,
                                stop=True,
                            )
                            et = p2.tile([P, QT], F32R, tag="et")
                            nc.scalar.activation(
                                et[:],
                                s_ps[:],
                                mybir.ActivationFunctionType.Exp,
                            )
                            nc.tensor.matmul(
                                outT_ps[:],
                                lhsT=v_b[b][:, kc],
                                rhs=et[:],
                                start=(kc == 0),
                                stop=(kc == KC - 1),
                            )
                        rs = p2.tile([1, QT], F32R, tag="rs")
                        with nc.allow_low_precision(reason="f32r recip scale"):
                            nc.vector.reciprocal(rs[:], outT_ps[DH : DH + 1, :])
                        bc_ps = psw.tile([DH, QT], F32, tag="bc")
                        nc.tensor.matmul(
                            bc_ps[:],
                            lhsT=ones1_t[:],
                            rhs=rs[:],
                            start=True,
                            stop=True,
                        )
                        bc_sb = p2.tile([DH, QT], F32, tag="bc_sb")
                        nc.vector.tensor_copy(bc_sb[:], bc_ps[:])
                        nc.vector.tensor_mul(
                            ho_t[0:DH, :],
                            outT_ps[0:DH, :],
                            bc_sb[:],
                        )
                        for sq in range(QT // P):
                            wo_ps = psw.tile([P, DIM], F32, tag="wo")
                            nc.tensor.matmul(
                                wo_ps[:],
                                lhsT=ho_t[:, sq * P : (sq + 1) * P],
                                rhs=wout_t[:],
                                start=True,
                                stop=True,
                            )
                            ob = p2.tile([P, DIM], F32, tag="ob")
                            nc.vector.tensor_copy(ob[:], wo_ps[:])
                            row0 = jq * QT + sq * P
                            nc.sync.dma_start(
                                out_d[b, row0 : row0 + P, :], ob[:]
                            )

    nc.compile()
    return nc


def _host_inputs(x, pos_bias, W_qkv, W_out):
    """Build the per-core input maps (pure data marshalling)."""
    xT = np.ascontiguousarray(x.transpose(0, 2, 1))          # [B, DIM, N]
    xT4 = np.ascontiguousarray(
        xT.reshape(B, DC, P, N).transpose(0, 2, 1, 3)
    )                                                        # [B, P, DC, N]

    inv_freq = (1.0 / (10000.0 ** (np.arange(0, DH, 2, dtype=np.float32) / DH)))
    freqs = np.arange(N, dtype=np.float32)[:, None] * inv_freq[None, :]  # [N, 32]
    cos_e = np.cos(freqs).astype(np.float32)
    sin_e = np.sin(freqs).astype(np.float32)
    cos_e = np.ascontiguousarray(
        cos_e.reshape(NCH, P, DH // 2).transpose(1, 0, 2)
    )
    sin_e = np.ascontiguousarray(
        sin_e.reshape(NCH, P, DH // 2).transpose(1, 0, 2)
    )
    ident = np.eye(P, dtype=np.float32)
    zpad = np.zeros((DH, N), dtype=np.float32)
    onesv = np.ones((P, KC), dtype=np.float32)
    ones1 = np.ones((1, DH), dtype=np.float32)

    scale = np.float32(DH ** -0.5)
    in_maps = []
    for h in range(HEADS):
        Wq = W_qkv[:, h * DH : (h + 1) * DH] * scale
        Wk = W_qkv[:, DIM + h * DH : DIM + (h + 1) * DH]
        Wv = W_qkv[:, 2 * DIM + h * DH : 2 * DIM + (h + 1) * DH]
        Wh = np.concatenate([Wq, Wk, Wv], axis=1).astype(np.float32)  # [512,192]
        w = np.ascontiguousarray(
            Wh.reshape(DC, P, 3 * DH).transpose(1, 0, 2)
        )
        wout = np.zeros((P, DIM), dtype=np.float32)
        wout[:DH] = W_out[h * DH : (h + 1) * DH, :]
        biasT = np.ascontiguousarray(pos_bias[h].T)
        in_maps.append(
            {
                "xT4": xT4,
                "w": w,
                "wout": wout,
                "biasT": biasT,
                "ident": ident,
                "zpad": zpad,
                "onesv": onesv,
                "ones1": ones1,
                "cos_e": cos_e,
                "sin_e": sin_e,
            }
        )
    return in_maps


def kernel(x, pos_bias, W_qkv, W_out, _trace=False):
    from concourse.bass_utils import run_bass_kernel_spmd

    x = np.asarray(x, dtype=np.float32)
    pos_bias = np.asarray(pos_bias, dtype=np.float32)
    W_qkv = np.asarray(W_qkv, dtype=np.float32)
    W_out = np.asarray(W_out, dtype=np.float32)

    if "nc" not in _CACHE:
        _CACHE["nc"] = _build()
    nc = _CACHE["nc"]

    in_maps = _host_inputs(x, pos_bias, W_qkv, W_out)
    try:
        res = run_bass_kernel_spmd(
            nc, in_maps, core_ids=list(range(HEADS)), trace=_trace
        )
    except ModuleNotFoundError:
        res = run_bass_kernel_spmd(
            nc, in_maps, core_ids=list(range(HEADS)), trace=False
        )
    out = np.zeros((B, N, DIM), dtype=np.float32)
    for rmap in res.results:
        out += rmap["out"]
    if _trace:
        return out, res
    return out


if __name__ == "__main__":
    rng = np.random.default_rng(0)
    x = rng.standard_normal((B, N, DIM), dtype=np.float32)
    pb = rng.standard_normal((HEADS, N, N), dtype=np.float32)
    wq = rng.standard_normal((DIM, 3 * DIM), dtype=np.float32) * DIM**-0.5
    wo = rng.standard_normal((DIM, DIM), dtype=np.float32) * DIM**-0.5
    o = kernel(x, pb, wq, wo)
    print("kernel ran, out std:", o.std())



# revision 4
# speedup vs baseline: 2.3372x; 2.3372x over previous
"""Attention kernel for trn2: B=4, N=2048, DIM=512, HEADS=8, DIM_HEAD=64.

Sharding: head-parallel across 8 cores (core h computes head h for all 4
batches). Each core returns a partial [4, 2048, 512] bf16 output (its head's
contribution through W_out); the host sums the 8 partials in fp32.

Per-core pipeline (all matmuls bf16, fp32 PSUM accumulate):
  phase 1 (projections, W-stationary so q/k emerge pre-transposed):
    QKc^T = [Wq|Wk]^T x^T and QKs^T = [Wq P|Wk P]^T x^T  (P = rotate-half
    permutation folded into the weights on host), then rotary is just
    rot = QKc*cos + QKs*sin on DVE (position runs along the free axis).
    v is projected x-stationary into natural [n, d] layout. DMA sbuf->sbuf
    remaps build qdup (q^T duplicated into both partition halves) and kTp
    (k^T chunks packed by parity into halves).
  phase 2 (attention, per (batch, 512-wide q tile)):
    S^T pairs via two concurrent K=64 row-tiled matmuls -> 2 psum banks;
    ACT exp over the [128,1024] pair (psum->sbuf bf16); attn = et * expB
    (host-precomputed exp(bias^T) bf16, loaded once per q-tile and shared
    by all 4 batches) on DVE/GpSimd; PV accumulates out^T (+ ones column
    for the softmax denominator); denominator is transposed via K=1
    matmuls to get per-partition reciprocals; W_out projection (K=64) with
    normalization folded into the psum evacuation as a tensor_scalar mul.
"""

import numpy as np

B, N, DIM = 4, 2048, 512
HEADS, DH = 8, 64
P = 128
DC = DIM // P          # 4 dim chunks
NCH = N // P           # 16 n chunks
QT = 512               # q tile in phase 2
NQT = N // QT          # 4
PAIRS = NCH // 2       # 8 k-chunk pairs
NB = N // QT           # 4 n blocks in phase 1
GPS_PAIRS = (2, 5)     # pairs whose bias-multiply runs on GpSimd

_CACHE = {}


def _build():
    import concourse.mybir as mybir
    import concourse.tile as tile
    from concourse import bacc

    F32 = mybir.dt.float32
    BF16 = mybir.dt.bfloat16
    EXP = mybir.ActivationFunctionType.Exp

    nc = bacc.Bacc(None, target_bir_lowering=False)

    # ---- inputs ----
    xT4_d = nc.dram_tensor("xT4", [B, P, DC, N], BF16, kind="ExternalInput")
    wqk_d = nc.dram_tensor("wqk", [P, 2, DC, P], BF16, kind="ExternalInput")
    wv_d = nc.dram_tensor("wv", [P, DC, DH], BF16, kind="ExternalInput")
    wout_d = nc.dram_tensor("wout", [DH, DIM], BF16, kind="ExternalInput")
    expb_d = nc.dram_tensor(
        "expb", [NQT, P, PAIRS, 2, QT], BF16, kind="ExternalInput"
    )
    cos2_d = nc.dram_tensor("cos2", [P, N], BF16, kind="ExternalInput")
    sin2_d = nc.dram_tensor("sin2", [P, N], BF16, kind="ExternalInput")
    onesv_d = nc.dram_tensor("onesv", [P, NCH], BF16, kind="ExternalInput")
    out_d = nc.dram_tensor("out", [B, N, DIM], BF16, kind="ExternalOutput")

    with tile.TileContext(nc) as tc:
        with tc.tile_pool(name="const", bufs=1) as cp:
            wqk_t = cp.tile([P, 2, DC, P], BF16, tag="wqk")
            nc.sync.dma_start(wqk_t[:], wqk_d[:, :, :, :])
            wv_t = cp.tile([P, DC, DH], BF16, tag="wv")
            nc.sync.dma_start(wv_t[:], wv_d[:, :, :])
            wout_t = cp.tile([DH, DIM], BF16, tag="wout")
            nc.sync.dma_start(wout_t[:], wout_d[:, :])
            cos2_t = cp.tile([P, N], BF16, tag="cos2")
            nc.sync.dma_start(cos2_t[:], cos2_d[:, :])
            sin2_t = cp.tile([P, N], BF16, tag="sin2")
            nc.sync.dma_start(sin2_t[:], sin2_d[:, :])
            ones_t = cp.tile([P, NCH], BF16, tag="ones")
            nc.sync.dma_start(ones_t[:], onesv_d[:, :])

            # persistent per-batch activations
            qdup_b = [cp.tile([P, N], BF16, tag=f"qdup{b}", name=f"qdup{b}") for b in range(B)]
            kTp_b = [cp.tile([P, PAIRS, P], BF16, tag=f"kTp{b}", name=f"kTp{b}") for b in range(B)]
            v_b = [cp.tile([P, NCH, DH + 1], BF16, tag=f"v{b}", name=f"v{b}") for b in range(B)]
            for b in range(B):
                nc.sync.dma_start(v_b[b][:, :, DH : DH + 1], onesv_d[:, :, None])

            # ---- phase 1: projections + rotary (no PE transposes) ----
            with (
                tc.tile_pool(name="p1", bufs=3) as p1,
                tc.tile_pool(name="ps_qk", bufs=2, space="PSUM") as ps_qk,
                tc.tile_pool(name="ps_v", bufs=2, space="PSUM") as ps_v,
            ):
                for b in range(B):
                    rot = p1.tile([P, N], BF16, tag="rot")
                    for nb in range(NB):
                        ns = slice(nb * QT, (nb + 1) * QT)
                        xblk = p1.tile([P, DC, QT], BF16, tag="xblk")
                        nc.sync.dma_start(xblk[:], xT4_d[b, :, :, ns])
                        qkc_ps = ps_qk.tile([P, QT], F32, tag="qkc")
                        qks_ps = ps_qk.tile([P, QT], F32, tag="qks")
                        for dc in range(DC):
                            nc.tensor.matmul(
                                qkc_ps[:],
                                lhsT=wqk_t[:, 0, dc],
                                rhs=xblk[:, dc],
                                start=(dc == 0),
                                stop=(dc == DC - 1),
                            )
                        for dc in range(DC):
                            nc.tensor.matmul(
                                qks_ps[:],
                                lhsT=wqk_t[:, 1, dc],
                                rhs=xblk[:, dc],
                                start=(dc == 0),
                                stop=(dc == DC - 1),
                            )
                        # v: x-stationary, natural layout, 4 chunks per block
                        vblk_ps = ps_v.tile([P, 4, DH], F32, tag="vblk")
                        for ci in range(4):
                            for dc in range(DC):
                                nc.tensor.matmul(
                                    vblk_ps[:, ci],
                                    lhsT=xblk[:, dc, ci * P : (ci + 1) * P],
                                    rhs=wv_t[:, dc],
                                    start=(dc == 0),
                                    stop=(dc == DC - 1),
                                )
                        # casts psum->sbuf bf16
                        qkc_sb = p1.tile([P, QT], BF16, tag="qkc_sb")
                        nc.vector.tensor_copy(qkc_sb[:], qkc_ps[:])
                        qks_sb = p1.tile([P, QT], BF16, tag="qks_sb")
                        nc.vector.tensor_copy(qks_sb[:], qks_ps[:])
                        nc.vector.tensor_copy(
                            v_b[b][:, nb * 4 : nb * 4 + 4, 0:DH], vblk_ps[:]
                        )
                        # rotary: rot = qkc*cos + qks*sin  (bf16, 2x mode)
                        m1 = p1.tile([P, QT], BF16, tag="m1")
                        nc.vector.tensor_mul(m1[:], qkc_sb[:], cos2_t[:, ns])
                        m2 = p1.tile([P, QT], BF16, tag="m2")
                        nc.vector.tensor_mul(m2[:], qks_sb[:], sin2_t[:, ns])
                        nc.vector.tensor_add(rot[:, ns], m1[:], m2[:])
                    # layout remaps via DMA (cross-partition moves)
                    nc.sync.dma_start(qdup_b[b][0:DH, :], rot[0:DH, :])
                    nc.sync.dma_start(qdup_b[b][DH:P, :], rot[0:DH, :])
                    r3 = rot.rearrange("p (pr two f) -> p pr two f", two=2, f=P)
                    nc.sync.dma_start(kTp_b[b][0:DH, :, :], r3[DH:P, :, 0, :])
                    nc.sync.dma_start(kTp_b[b][DH:P, :, :], r3[DH:P, :, 1, :])

            # ---- phase 2: attention ----
            with (
                tc.tile_pool(name="eb", bufs=2) as ebp,
                tc.tile_pool(name="p2", bufs=3) as p2,
                tc.tile_pool(name="ps_s", bufs=2, space="PSUM") as ps_s,
                tc.tile_pool(name="ps_o", bufs=1, space="PSUM") as ps_o,
                tc.tile_pool(name="ps_w", bufs=2, space="PSUM") as ps_w,
                tc.tile_pool(name="ps_d", bufs=1, space="PSUM") as ps_d,
            ):
                for jq in range(NQT):
                    qs = slice(jq * QT, (jq + 1) * QT)
                    eb_t = ebp.tile([P, PAIRS, 2, QT], BF16, tag="eb")
                    nc.sync.dma_start(eb_t[:], expb_d[jq])
                    for b in range(B):
                        outT_ps = ps_o.tile([DH + 1, QT], F32, tag="outT")
                        for pr in range(PAIRS):
                            s_ps = ps_s.tile([P, 2, QT], F32, tag="s")
                            nc.tensor.matmul(
                                s_ps[:, 0],
                                lhsT=kTp_b[b][0:DH, pr],
                                rhs=qdup_b[b][0:DH, qs],
                                start=True,
                                stop=True,
                                tile_position=(0, 0),
                            )
                            nc.tensor.matmul(
                                s_ps[:, 1],
                                lhsT=kTp_b[b][DH:P, pr],
                                rhs=qdup_b[b][DH:P, qs],
                                start=True,
                                stop=True,
                                tile_position=(64, 0),
                            )
                            et = p2.tile([P, 2, QT], BF16, tag="et")
                            nc.scalar.activation(et[:], s_ps[:], EXP)
                            attn = p2.tile([P, 2, QT], BF16, tag="attn")
                            eng = nc.gpsimd if pr in GPS_PAIRS else nc.vector
                            eng.tensor_mul(attn[:], et[:], eb_t[:, pr])
                            nc.tensor.matmul(
                                outT_ps[:],
                                lhsT=v_b[b][:, 2 * pr],
                                rhs=attn[:, 0],
                                start=(pr == 0),
                                stop=False,
                            )
                            nc.tensor.matmul(
                                outT_ps[:],
                                lhsT=v_b[b][:, 2 * pr + 1],
                                rhs=attn[:, 1],
                                start=False,
                                stop=(pr == PAIRS - 1),
                            )
                        # denominator -> per-partition reciprocal
                        drow = p2.tile([DH + 1, QT], BF16, tag="drow")
                        nc.vector.tensor_copy(
                            drow[DH : DH + 1, :], outT_ps[DH : DH + 1, :]
                        )
                        dT_ps = ps_d.tile([P, 4], F32, tag="dT")
                        for s4 in range(4):
                            nc.tensor.matmul(
                                dT_ps[:, s4 : s4 + 1],
                                lhsT=drow[DH : DH + 1, s4 * P : (s4 + 1) * P],
                                rhs=ones_t[DH : DH + 1, 0:1],
                                start=True,
                                stop=True,
                            )
                        rs = p2.tile([P, 4], F32, tag="rs")
                        with nc.allow_low_precision(reason="softmax recip"):
                            nc.vector.reciprocal(rs[:], dT_ps[:])
                        ho = p2.tile([DH, QT], BF16, tag="ho")
                        nc.vector.tensor_copy(ho[:], outT_ps[0:DH, :])
                        for sq in range(4):
                            wo_ps = ps_w.tile([P, DIM], F32, tag="wo")
                            nc.tensor.matmul(
                                wo_ps[:],
                                lhsT=ho[:, sq * P : (sq + 1) * P],
                                rhs=wout_t[:],
                                start=True,
                                stop=True,
                            )
                            ob = p2.tile([P, DIM], BF16, tag="ob")
                            nc.vector.tensor_scalar_mul(
                                ob[:], wo_ps[:], rs[:, sq : sq + 1]
                            )
                            row0 = jq * QT + sq * P
                            nc.sync.dma_start(out_d[b, row0 : row0 + P, :], ob[:])

    nc.compile()
    return nc


def _host_inputs(x, pos_bias, W_qkv, W_out):
    """Build the per-core input maps (pure data marshalling)."""
    import ml_dtypes

    bf16 = ml_dtypes.bfloat16

    xT = np.ascontiguousarray(x.transpose(0, 2, 1))          # [B, DIM, N]
    xT4 = np.ascontiguousarray(
        xT.reshape(B, DC, P, N).transpose(0, 2, 1, 3)
    ).astype(bf16)                                           # [B, P, DC, N]

    # split-d permutation: even dims then odd dims
    perm = np.concatenate([np.arange(0, DH, 2), np.arange(1, DH, 2)])
    inv_freq = (1.0 / (10000.0 ** (np.arange(0, DH, 2, dtype=np.float32) / DH)))
    pos = np.arange(N, dtype=np.float32)
    fr = inv_freq[:, None] * pos[None, :]                     # [32, N]
    cos_h = np.cos(fr)
    sin_h = np.sin(fr)
    # rows: q-even, q-odd, k-even, k-odd halves all share the per-pair angle
    cos2 = np.concatenate([cos_h] * 4, axis=0).astype(bf16)   # [128, N]
    sin2 = np.concatenate([sin_h] * 4, axis=0).astype(bf16)

    onesv = np.ones((P, NCH), dtype=np.float32).astype(bf16)

    scale = np.float32(DH ** -0.5)
    in_maps = []
    for h in range(HEADS):
        Wq = (W_qkv[:, h * DH : (h + 1) * DH] * scale)[:, perm]   # split-d
        Wk = W_qkv[:, DIM + h * DH : DIM + (h + 1) * DH][:, perm]
        Wv = W_qkv[:, 2 * DIM + h * DH : 2 * DIM + (h + 1) * DH]
        # rotate-half in split layout: s_e = -c_o, s_o = c_e
        Wq_s = np.concatenate([-Wq[:, 32:64], Wq[:, 0:32]], axis=1)
        Wk_s = np.concatenate([-Wk[:, 32:64], Wk[:, 0:32]], axis=1)
        Wc = np.concatenate([Wq, Wk], axis=1)                 # [512, 128]
        Ws = np.concatenate([Wq_s, Wk_s], axis=1)             # [512, 128]
        wqk = np.ascontiguousarray(
            np.stack(
                [
                    Wc.reshape(DC, P, P).transpose(1, 0, 2),
                    Ws.reshape(DC, P, P).transpose(1, 0, 2),
                ],
                axis=1,
            )
        ).astype(bf16)                                        # [P, 2, DC, P]
        wv = np.ascontiguousarray(
            Wv.reshape(DC, P, DH).transpose(1, 0, 2)
        ).astype(bf16)                                        # [P, DC, DH]
        wout = np.ascontiguousarray(W_out[h * DH : (h + 1) * DH, :]).astype(bf16)
        ebT = np.exp(pos_bias[h].T.astype(np.float32))        # [k, q]
        expb = np.ascontiguousarray(
            ebT.reshape(PAIRS, 2, P, NQT, QT).transpose(3, 2, 0, 1, 4)
        ).astype(bf16)                                        # [NQT, P, PAIRS, 2, QT]
        in_maps.append(
            {
                "xT4": xT4,
                "wqk": wqk,
                "wv": wv,
                "wout": wout,
                "expb": expb,
                "cos2": cos2,
                "sin2": sin2,
                "onesv": onesv,
            }
        )
    return in_maps


def kernel(x, pos_bias, W_qkv, W_out, _trace=False):
    from concourse.bass_utils import run_bass_kernel_spmd

    x = np.asarray(x, dtype=np.float32)
    pos_bias = np.asarray(pos_bias, dtype=np.float32)
    W_qkv = np.asarray(W_qkv, dtype=np.float32)
    W_out = np.asarray(W_out, dtype=np.float32)

    if "nc" not in _CACHE:
        _CACHE["nc"] = _build()
    nc = _CACHE["nc"]

    in_maps = _host_inputs(x, pos_bias, W_qkv, W_out)
    try:
        res = run_bass_kernel_spmd(
            nc, in_maps, core_ids=list(range(HEADS)), trace=_trace
        )
    except ModuleNotFoundError:
        res = run_bass_kernel_spmd(
            nc, in_maps, core_ids=list(range(HEADS)), trace=False
        )
    out = np.zeros((B, N, DIM), dtype=np.float32)
    for rmap in res.results:
        out += rmap["out"].astype(np.float32)
    if _trace:
        return out, res
    return out


if __name__ == "__main__":
    rng = np.random.default_rng(0)
    x = rng.standard_normal((B, N, DIM), dtype=np.float32)
    pb = rng.standard_normal((HEADS, N, N), dtype=np.float32)
    wq = rng.standard_normal((DIM, 3 * DIM), dtype=np.float32) * DIM**-0.5
    wo = rng.standard_normal((DIM, DIM), dtype=np.float32) * DIM**-0.5
    o = kernel(x, pb, wq, wo)
    print("kernel ran, out std:", o.std())
